# revision 19
# baseline (speedup 1.0000x reference)
"""AdaptiveRegionGenerator on 8 trn2 NeuronCores (Bass/Tile SPMD).

Only the CLS row of the encoder output is needed, so the padded-sequence
encoder collapses to a segment-softmax attention over all patches with a
shared CLS query, plus a 64-row FFN. Patches are sharded 8x (2048/core);
four small AllReduces stitch the cores:
  #0  global T min/max           (16 B,  max of negated mins)
  #1  G_sum | counts             (64x769 f32, add)
  #1b per-region max dist        (64 f32, max)
  #2  attention numerator | Z    (64x776 f32, add)
Decision-critical matmuls (G_sum, C = F @ Gn^T) run true fp32; bulk matmuls
(K/V projection, segment sums, out-proj, FFN) run fp32r/bf16. The routing
decisions (bin edges, topk-3, argmax) replicate the reference's fp32
arithmetic exactly (linspace edges, true divides, exact 1-x negation).
"""
import sys

sys.path.insert(0, "/opt/trn_rl_repo")

import numpy as np

N = 16384
D = 768
DFF = 2048
RR = 64
NH = 8
HD = 96
NCORE = 8
NLOC = N // NCORE          # 2048 patches per core
NCH = NLOC // 128          # 16 chunks of 128
NCH_A = NCH + 1            # +1 chunk holding the cls row at partition 0
SC96 = float(1.0 / np.float32(np.sqrt(np.float64(HD))))

_CACHE = {}
GELU_SIM = False   # sim lacks Gelu: use tanh-approx there (wiring check only)


def _build():
    from concourse import bass, bacc, tile, mybir, library_config

    f32 = mybir.dt.float32
    f32r = mybir.dt.float32r
    bf16 = mybir.dt.bfloat16
    Alu = mybir.AluOpType
    Act = mybir.ActivationFunctionType
    AX = mybir.AxisListType.X

    nc = bacc.Bacc(None)
    RG = [list(range(NCORE))]

    f_in = nc.declare_dram_parameter("f", [NLOC, D], f32, isOutput=False)
    ft_in = nc.declare_dram_parameter("ft", [D, 128 * NCH_A], f32, isOutput=False)
    tsh_in = nc.declare_dram_parameter("tsh", [128, 2, NCH], f32, isOutput=False)
    wqkv_in = nc.declare_dram_parameter("wqkv", [D, 3 * D], f32, isOutput=False)
    wo_in = nc.declare_dram_parameter("wo", [D, D], f32, isOutput=False)
    l1_in = nc.declare_dram_parameter("l1", [D, DFF], f32, isOutput=False)
    l2_in = nc.declare_dram_parameter("l2", [DFF, D], f32, isOutput=False)
    bqkv_in = nc.declare_dram_parameter("bqkv", [1, 3 * D], f32, isOutput=False)
    b1c_in = nc.declare_dram_parameter("b1c", [128, NCH], f32, isOutput=False)
    vecs_in = nc.declare_dram_parameter("vecs", [1, 7 * D], f32, isOutput=False)
    ident_in = nc.declare_dram_parameter("ident", [128, 128], f32, isOutput=False)
    consts_in = nc.declare_dram_parameter("consts", [128, 2, RR], f32, isOutput=False)
    out_p = nc.declare_dram_parameter("out", [RR, D], f32, isOutput=True)
    ri_p = nc.declare_dram_parameter("ri", [128, NCH], f32, isOutput=True)

    cc0_i = nc.dram_tensor("cc0_i", [4, 1], f32)
    cc0_o = nc.dram_tensor("cc0_o", [4, 1], f32, addr_space="Shared")
    cc1_i = nc.dram_tensor("cc1_i", [RR, D + 1], f32)
    cc1_o = nc.dram_tensor("cc1_o", [RR, D + 1], f32, addr_space="Shared")
    ccm_i = nc.dram_tensor("ccm_i", [RR, 1], f32)
    ccm_o = nc.dram_tensor("ccm_o", [RR, 1], f32, addr_space="Shared")
    cc2_i = nc.dram_tensor("cc2_i", [RR, D + NH], f32)
    cc2_o = nc.dram_tensor("cc2_o", [RR, D + NH], f32, addr_space="Shared")

    with tile.TileContext(nc) as tc:
        with (
            tc.tile_pool(name="persist", bufs=1) as pp,
            tc.tile_pool(name="fstream", bufs=2) as fs,
            tc.tile_pool(name="small", bufs=1) as sm,
            tc.tile_pool(name="psbig", bufs=1, space="PSUM") as psb,
            tc.tile_pool(name="psg", bufs=1, space="PSUM") as psg,
            tc.tile_pool(name="pss", bufs=2, space="PSUM") as pss,
        ):
            dma = nc.sync.dma_start
            nc.gpsimd.load_library(library_config.mlp)

            def ft_tile(d, j, dt=None):
                t = fs.tile([128, 128], dt or f32, tag=f"ftd{d}")
                srcap = ft_in[d * 128:(d + 1) * 128, j * 128:(j + 1) * 128]
                if dt is f32r:
                    srcap = srcap.bitcast(f32r)
                dma(out=t[:], in_=srcap)
                return t

            # ---------- persistent loads ----------
            wq = []
            for d in range(6):
                t = pp.tile([128, 3 * D], f32r, tag=f"wq{d}")
                dma(out=t[:], in_=wqkv_in[d * 128:(d + 1) * 128, :].bitcast(f32r))
                wq.append(t)
            tsh = pp.tile([128, 2, NCH], f32, tag="tsh")
            dma(out=tsh[:], in_=tsh_in[:])
            ident = pp.tile([128, 128], f32, tag="ident")
            dma(out=ident[:], in_=ident_in[:])
            consts = pp.tile([128, 2, RR], f32, tag="consts")
            dma(out=consts[:], in_=consts_in[:])
            bqkv = pp.tile([1, 3 * D], f32, tag="bqkv")
            dma(out=bqkv[:], in_=bqkv_in[:])
            vecs = pp.tile([1, 7 * D], f32, tag="vecs")
            dma(out=vecs[:], in_=vecs_in[:])
            b1c = pp.tile([128, NCH], f32, tag="b1c")
            dma(out=b1c[:], in_=b1c_in[:])

            iota_bc = consts[:, 0, :]
            iom_bc = consts[:, 1, :]

            # ---------- collective #0: global T min/max ----------
            tminp = sm.tile([128, 2], f32, tag="tminp")
            tmaxp = sm.tile([128, 2], f32, tag="tmaxp")
            nc.vector.tensor_reduce(tminp[:], tsh[:], AX, Alu.min)
            nc.vector.tensor_reduce(tmaxp[:], tsh[:], AX, Alu.max)
            pack4 = sm.tile([128, 4], f32, tag="pack4")
            nc.vector.tensor_copy(pack4[:, 0:2], tmaxp[:])
            nc.vector.tensor_scalar(pack4[:, 2:4], tminp[:], -1.0, None, Alu.mult)
            ps4 = pss.tile([4, 128], f32, tag="ps_sm")
            nc.tensor.transpose(ps4[:], pack4[:], ident[:])
            red4 = sm.tile([4, 1], f32, tag="red4")
            nc.vector.tensor_reduce(red4[:], ps4[:], AX, Alu.max)
            dma(out=cc0_i[:], in_=red4[:])
            nc.gpsimd.collective_compute(
                "AllReduce", Alu.max, replica_groups=RG,
                ins=[cc0_i[:]], outs=[cc0_o[:]],
            )
            mm4 = sm.tile([4, 1], f32, tag="mm4")
            dma(out=mm4[:], in_=cc0_o[:])
            # move the 4 values onto one partition row
            ps4b = pss.tile([1, 4], f32, tag="ps_sm")
            nc.tensor.transpose(ps4b[:], mm4[:, 0:1], ident[0:4, 0:4])
            mrow4 = sm.tile([1, 4], f32, tag="mrow4")
            nc.vector.tensor_copy(mrow4[:], ps4b[:])

            # edges: start=tmin, span=max(tmax-tmin,eps), stop=start+span,
            # step=(stop-start)*0.125, e_j = start + j*step (bit-exact linspace)
            tmn = sm.tile([1, 2], f32, tag="tmn")
            nc.vector.tensor_scalar(tmn[:], mrow4[:, 2:4], -1.0, None, Alu.mult)
            spn = sm.tile([1, 2], f32, tag="spn")
            nc.vector.tensor_tensor(spn[:], mrow4[:, 0:2], tmn[:], Alu.subtract)
            eps32 = float(np.finfo(np.float32).eps)
            nc.vector.tensor_scalar(spn[:], spn[:], eps32, None, Alu.max)
            stp = sm.tile([1, 2], f32, tag="stp")
            nc.vector.tensor_tensor(stp[:], tmn[:], spn[:], Alu.add)       # stop
            nc.vector.tensor_tensor(stp[:], stp[:], tmn[:], Alu.subtract)  # -start
            nc.vector.tensor_scalar(stp[:], stp[:], 0.125, None, Alu.mult)
            exy = sm.tile([1, 2, 9], f32, tag="exy")
            for ax in range(2):
                nc.vector.tensor_scalar(
                    exy[:, ax, :], consts[0:1, 0, 0:9],
                    stp[:, ax:ax + 1], tmn[:, ax:ax + 1], Alu.mult, Alu.add)
            anc = sm.tile([1, 2, 8], f32, tag="anc")
            nc.vector.tensor_tensor(anc[:], exy[:, :, 0:8], exy[:, :, 1:9], Alu.add)
            nc.vector.tensor_scalar(
                anc[:].rearrange("a b c -> a (b c)"),
                anc[:].rearrange("a b c -> a (b c)"), 0.5, None, Alu.mult)
            prowx = sm.tile([1, RR], f32, tag="prowx")
            prowy = sm.tile([1, RR], f32, tag="prowy")
            nc.vector.tensor_copy(
                prowx[:].rearrange("a (o i) -> a o i", o=8),
                anc[:, 0, :].rearrange("a (e o) -> a e o", o=1).broadcast_to([1, 8, 8]))
            nc.vector.tensor_copy(
                prowy[:].rearrange("a (o i) -> a o i", o=8),
                anc[:, 1, :].rearrange("a (o e) -> a o e", o=1).broadcast_to([1, 8, 8]))
            erow = sm.tile([1, 14], f32, tag="erow")
            nc.vector.tensor_copy(
                erow[:].rearrange("a (b c) -> a b c", b=2), exy[:, :, 1:8])
            edg_bc = pp.tile([128, 2, 7], f32, tag="edg_bc")
            nc.gpsimd.partition_broadcast(
                edg_bc[:].rearrange("p a e -> p (a e)"), erow[:])
            px_bc = pp.tile([128, RR], f32, tag="px_bc")
            py_bc = pp.tile([128, RR], f32, tag="py_bc")
            nc.gpsimd.partition_broadcast(px_bc[:], prowx[:])
            nc.gpsimd.partition_broadcast(py_bc[:], prowy[:])

            # ---------- cls chunk projections (Q then KV) ----------
            ps_q = psb.tile([128, D], f32, tag="psbig")
            for d in range(6):
                lt = ft_tile(d, NCH, f32r)
                nc.tensor.matmul(ps_q[:, 0:512], lt[:], wq[d][:, 0:512],
                                 start=(d == 0), stop=(d == 5))
                nc.tensor.matmul(ps_q[:, 512:768], lt[:], wq[d][:, 512:768],
                                 start=(d == 0), stop=(d == 5))
            qrow = pp.tile([1, D], f32, tag="qrow")
            nc.vector.tensor_tensor(qrow[:], ps_q[0:1, :], bqkv[:, 0:D], Alu.add)
            ps_kv = psb.tile([128, 2 * D], f32, tag="psbig")
            for d in range(6):
                lt = ft_tile(d, NCH, f32r)
                for s in range(3):
                    nc.tensor.matmul(
                        ps_kv[:, s * 512:(s + 1) * 512], lt[:],
                        wq[d][:, D + s * 512:D + (s + 1) * 512],
                        start=(d == 0), stop=(d == 5))
            krow = sm.tile([1, D], f32, tag="rowA")
            vrow = pp.tile([1, D], f32, tag="vrow")
            nc.vector.tensor_tensor(krow[:], ps_kv[0:1, 0:D], bqkv[:, D:2 * D], Alu.add)
            nc.vector.tensor_tensor(vrow[:], ps_kv[0:1, D:2 * D], bqkv[:, 2 * D:], Alu.add)
            qk = sm.tile([1, D], f32, tag="qk")
            nc.vector.tensor_tensor(qk[:], qrow[:], krow[:], Alu.mult)
            lcls = sm.tile([1, NH], f32, tag="lcls")
            nc.vector.tensor_reduce(
                lcls[:], qk[:].rearrange("a (h e) -> a h e", h=NH), AX, Alu.add)
            elcls = pp.tile([1, NH], f32, tag="elcls")
            nc.scalar.activation(elcls[:], lcls[:], Act.Exp, scale=SC96)
            qbk = sm.tile([1, D], f32, tag="rowA")
            nc.vector.tensor_tensor(qbk[:], qrow[:], bqkv[:, D:2 * D], Alu.mult)
            kbr = sm.tile([1, NH], f32, tag="kbr")
            nc.vector.tensor_reduce(
                kbr[:], qbk[:].rearrange("a (h e) -> a h e", h=NH), AX, Alu.add)
            kb_bc = pp.tile([128, NH], f32, tag="kb_bc")
            nc.gpsimd.partition_broadcast(kb_bc[:], kbr[:])
            qcls_bc = pp.tile([128, D], f32, tag="qcls_bc")
            nc.gpsimd.partition_broadcast(qcls_bc[:], qrow[:])

            # ---------- K/V projection + l/el/u per patch chunk ----------
            u_t = pp.tile([128, NCH, D], bf16, tag="u")
            el_t = pp.tile([128, NCH, NH], f32, tag="el")
            elb_t = pp.tile([128, NCH, NH], bf16, tag="elb")
            for j in range(NCH):
                ps = psb.tile([128, 2 * D], f32, tag="psbig")
                for d in range(6):
                    lt = ft_tile(d, j, f32r)
                    for s in range(3):
                        nc.tensor.matmul(
                            ps[:, s * 512:(s + 1) * 512], lt[:],
                            wq[d][:, D + s * 512:D + (s + 1) * 512],
                            start=(d == 0), stop=(d == 5))
                lmul = fs.tile([128, D], f32, tag="lmul")
                nc.vector.tensor_tensor(lmul[:], ps[:, 0:D], qcls_bc[:], Alu.mult)
                lraw = fs.tile([128, NH], f32, tag="lraw")
                nc.vector.tensor_reduce(
                    lraw[:], lmul[:].rearrange("p (h e) -> p h e", h=NH), AX, Alu.add)
                nc.vector.tensor_tensor(lraw[:], lraw[:], kb_bc[:], Alu.add)
                nc.scalar.activation(el_t[:, j, :], lraw[:], Act.Exp, scale=SC96)
                nc.vector.tensor_copy(elb_t[:, j, :], el_t[:, j, :])
                nc.vector.tensor_tensor(
                    u_t[:, j, :].rearrange("p (h e) -> p h e", h=NH),
                    ps[:, D:2 * D].rearrange("p (h e) -> p h e", h=NH),
                    el_t[:, j, :].rearrange("p (h e) -> p h e", e=1)
                        .broadcast_to([128, NH, HD]),
                    Alu.mult)

            # ---------- bins, d2, G_sum (needs #0 results) ----------
            oh0_t = pp.tile([128, NCH, RR], f32, tag="oh0")
            d2_t = pp.tile([128, NCH, RR], f32, tag="d2")
            fs2 = pp.tile([128, NCH], f32, tag="fs2")
            ps_g = psg.tile([RR, D], f32, tag="psg")
            ps_gc = psg.tile([RR, 1], f32, tag="psgc")
            ones_c = pp.tile([128, 1], f32, tag="ones_c")
            nc.vector.memset(ones_c[:], 1.0)
            for j in range(NCH):
                xb = fs.tile([128, 2, 7], f32, tag="xb")
                for ax in range(2):
                    nc.vector.tensor_scalar(
                        xb[:, ax, :], edg_bc[:, ax, :],
                        tsh[:, ax, j:j + 1], None, Alu.is_lt)
                binc = fs.tile([128, 2], f32, tag="binc")
                nc.vector.tensor_reduce(binc[:], xb[:], AX, Alu.add)
                init = fs.tile([128, 1], f32, tag="init")
                nc.vector.tensor_scalar(
                    init[:], binc[:, 0:1], 8.0, binc[:, 1:2], Alu.mult, Alu.add)
                nc.vector.tensor_scalar(
                    oh0_t[:, j, :], iota_bc[:], init[:], None, Alu.is_equal)
                dx = fs.tile([128, RR], f32, tag="dx")
                nc.vector.tensor_scalar(dx[:], px_bc[:], tsh[:, 0, j:j + 1],
                                        None, Alu.subtract)
                nc.vector.tensor_tensor(dx[:], dx[:], dx[:], Alu.mult)
                dy = fs.tile([128, RR], f32, tag="dy")
                nc.vector.tensor_scalar(dy[:], py_bc[:], tsh[:, 1, j:j + 1],
                                        None, Alu.subtract)
                nc.vector.tensor_tensor(dy[:], dy[:], dy[:], Alu.mult)
                nc.vector.tensor_tensor(d2_t[:, j, :], dx[:], dy[:], Alu.add)
                fj = fs.tile([128, D], f32, tag="fj")
                dma(out=fj[:], in_=f_in[j * 128:(j + 1) * 128, :])
                sq_scr = fs.tile([128, D], f32, tag="sq_scr")
                nc.scalar.activation(sq_scr[:], fj[:], Act.Square,
                                     accum_out=fs2[:, j:j + 1])
                nc.tensor.matmul(ps_g[:, 0:512], oh0_t[:, j, :], fj[:, 0:512],
                                 start=(j == 0), stop=(j == NCH - 1))
                nc.tensor.matmul(ps_g[:, 512:768], oh0_t[:, j, :], fj[:, 512:768],
                                 start=(j == 0), stop=(j == NCH - 1))
                nc.tensor.matmul(ps_gc[:], oh0_t[:, j, :], ones_c[:],
                                 start=(j == 0), stop=(j == NCH - 1))
            gsum_s = sm.tile([RR, D + 1], f32, tag="r64")
            nc.vector.tensor_copy(gsum_s[:, 0:D], ps_g[:])
            nc.vector.tensor_copy(gsum_s[:, D:D + 1], ps_gc[:])
            dma(out=cc1_i[:], in_=gsum_s[:])
            nc.gpsimd.collective_compute(
                "AllReduce", Alu.add, replica_groups=RG,
                ins=[cc1_i[:]], outs=[cc1_o[:]],
            )

            # ---------- per-region max dist^2 (collective #1b) ----------
            mdr = sm.tile([128, RR], f32, tag="mdr")
            nc.vector.tensor_copy(mdr[:], d2_t[:, 0, :])
            for j in range(1, NCH):
                nc.vector.tensor_tensor(mdr[:], mdr[:], d2_t[:, j, :], Alu.max)
            ps_t = pss.tile([RR, 128], f32, tag="ps_sm")
            nc.tensor.transpose(ps_t[:], mdr[:], ident[:])
            md64 = sm.tile([RR, 1], f32, tag="md64")
            nc.vector.tensor_reduce(md64[:], ps_t[:], AX, Alu.max)
            dma(out=ccm_i[:], in_=md64[:])
            nc.gpsimd.collective_compute(
                "AllReduce", Alu.max, replica_groups=RG,
                ins=[ccm_i[:]], outs=[ccm_o[:]],
            )

            # ---------- G -> Gn -> Gn^T ----------
            g_s = sm.tile([RR, D + 1], f32, tag="r64")
            dma(out=g_s[:], in_=cc1_o[:])
            cntm = sm.tile([RR, 1], f32, tag="cntm")
            nc.vector.tensor_scalar(cntm[:], g_s[:, D:D + 1], 1.0, None, Alu.max)
            nc.vector.reciprocal(cntm[:], cntm[:])
            gn = sm.tile([RR, D], f32, tag="v64a")
            nc.vector.tensor_scalar(gn[:], g_s[:, 0:D], cntm[:], None, Alu.mult)
            gsq = sm.tile([RR, D], f32, tag="sq64")
            gss = sm.tile([RR, 1], f32, tag="gss")
            nc.scalar.activation(gsq[:], gn[:], Act.Square, accum_out=gss[:])
            nc.scalar.activation(gss[:], gss[:], Act.Sqrt)
            nc.vector.tensor_scalar(gss[:], gss[:], 1e-8, None, Alu.max)
            nc.vector.reciprocal(gss[:], gss[:])
            nc.vector.tensor_scalar(gn[:], gn[:], gss[:], None, Alu.mult)
            gnT = []
            for d in range(6):
                pt = pss.tile([128, RR], f32, tag="ps_sm")
                nc.tensor.transpose(pt[:], gn[:, d * 128:(d + 1) * 128],
                                    ident[0:RR, 0:RR])
                t = pp.tile([128, RR], f32, tag=f"gnT{d}")
                nc.vector.tensor_copy(t[:], pt[:])
                gnT.append(t)

            # ---------- mu = 1 - dist/maxd (in place over d2) ----------
            mrow = sm.tile([1, RR], f32, tag="mrow")
            dma(out=mrow[:], in_=ccm_o[:])
            nc.scalar.activation(mrow[:], mrow[:], Act.Sqrt)
            nc.vector.tensor_scalar(mrow[:], mrow[:], 1e-12, None, Alu.max)
            nc.vector.reciprocal(mrow[:], mrow[:])
            md_bc = pp.tile([128, RR], f32, tag="md_bc")
            nc.gpsimd.partition_broadcast(md_bc[:], mrow[:])
            nc.scalar.activation(
                d2_t[:].rearrange("p a b -> p (a b)"),
                d2_t[:].rearrange("p a b -> p (a b)"), Act.Sqrt)
            mu8 = pp.tile([128, NCH, 8], f32, tag="mu8")
            musel = pp.tile([128, NCH, RR], f32, tag="musel")
            for j in range(NCH):
                nc.vector.tensor_tensor(d2_t[:, j, :], d2_t[:, j, :], md_bc[:],
                                        Alu.mult)
                nc.vector.tensor_scalar(
                    d2_t[:, j, :], d2_t[:, j, :], 1.0, -1.0, Alu.subtract, Alu.mult)
                nc.vector.max(mu8[:, j, :], d2_t[:, j, :])
                sel = fs.tile([128, RR], f32, tag="sel")
                nc.vector.tensor_scalar(
                    sel[:], d2_t[:, j, :], mu8[:, j, 2:3], None, Alu.is_ge)
                nc.vector.tensor_tensor(musel[:, j, :], d2_t[:, j, :], sel[:],
                                        Alu.mult)

            # ---------- C = F @ Gn^T (fp32), decisions, one-hots ----------
            fnrm = pp.tile([128, NCH], f32, tag="fnrm")
            nc.scalar.activation(fnrm[:], fs2[:], Act.Sqrt)
            nc.vector.tensor_scalar(fnrm[:], fnrm[:], 1e-8, None, Alu.max)
            nc.vector.reciprocal(fnrm[:], fnrm[:])
            asg = pp.tile([128, NCH], f32, tag="asg")
            ohb_t = pp.tile([128, NCH, RR], bf16, tag="ohb")
            for j in range(NCH):
                pc = pss.tile([128, RR], f32, tag="ps_sm")
                for d in range(6):
                    lt = ft_tile(d, j)
                    nc.tensor.matmul(pc[:], lt[:], gnT[d][:],
                                     start=(d == 0), stop=(d == 5))
                wj = fs.tile([128, RR], f32, tag="wj")
                nc.vector.tensor_scalar(wj[:], pc[:], fnrm[:, j:j + 1], None,
                                        Alu.mult)
                nc.scalar.activation(wj[:], wj[:], Act.Exp)
                nc.vector.tensor_tensor(wj[:], wj[:], musel[:, j, :], Alu.mult)
                mj = fs.tile([128, 1], f32, tag="mj")
                nc.vector.tensor_reduce(mj[:], wj[:], AX, Alu.max)
                nc.vector.tensor_scalar(wj[:], wj[:], mj[:], None, Alu.is_equal)
                nc.vector.tensor_tensor(wj[:], wj[:], iom_bc[:], Alu.mult)
                nc.vector.tensor_scalar(wj[:], wj[:], 65536.0, None, Alu.add)
                nc.vector.tensor_reduce(asg[:, j:j + 1], wj[:], AX, Alu.min)
                nc.vector.tensor_scalar(ohb_t[:, j, :], iota_bc[:],
                                        asg[:, j:j + 1], None, Alu.is_equal)
            dma(out=ri_p[:], in_=asg[:])

            # ---------- segment sums ----------
            ps_a = psg.tile([RR, D], f32, tag="psg")
            ps_az = psg.tile([RR, NH], f32, tag="psgc")
            for j in range(NCH):
                nc.tensor.matmul(ps_a[:, 0:512], ohb_t[:, j, :], u_t[:, j, 0:512],
                                 start=(j == 0), stop=(j == NCH - 1))
                nc.tensor.matmul(ps_a[:, 512:768], ohb_t[:, j, :],
                                 u_t[:, j, 512:768],
                                 start=(j == 0), stop=(j == NCH - 1))
                nc.tensor.matmul(ps_az[:], ohb_t[:, j, :],
                                 elb_t[:, j, :],
                                 start=(j == 0), stop=(j == NCH - 1))
            seg_s = sm.tile([RR, D + NH], f32, tag="r64")
            nc.vector.tensor_copy(seg_s[:, 0:D], ps_a[:])
            nc.vector.tensor_copy(seg_s[:, D:D + NH], ps_az[:])
            dma(out=cc2_i[:], in_=seg_s[:])
            nc.gpsimd.collective_compute(
                "AllReduce", Alu.add, replica_groups=RG,
                ins=[cc2_i[:]], outs=[cc2_o[:]],
            )

            # ---------- finale (replicated on all cores) ----------
            red = sm.tile([RR, D + NH], f32, tag="r64")
            dma(out=red[:], in_=cc2_o[:])
            elc_bc = sm.tile([RR, NH], f32, tag="elc_bc")
            nc.gpsimd.partition_broadcast(elc_bc[:], elcls[:])
            ztot = sm.tile([RR, NH], f32, tag="ztot")
            nc.vector.tensor_tensor(ztot[:], red[:, D:D + NH], elc_bc[:], Alu.add)
            evc = sm.tile([1, D], f32, tag="rowA")
            for h in range(NH):
                nc.vector.tensor_scalar(
                    evc[:, h * HD:(h + 1) * HD], vrow[:, h * HD:(h + 1) * HD],
                    elcls[:, h:h + 1], None, Alu.mult)
            bc_a = sm.tile([RR, D], f32, tag="bc_a")
            nc.gpsimd.partition_broadcast(bc_a[:], evc[:])
            o_t = sm.tile([RR, D], f32, tag="v64a")
            nc.vector.tensor_tensor(o_t[:], red[:, 0:D], bc_a[:], Alu.add)
            nc.gpsimd.partition_broadcast(bc_a[:], bqkv[:, 2 * D:])   # b_v
            zrec = sm.tile([RR, NH], f32, tag="zrec")
            nc.vector.reciprocal(zrec[:], ztot[:])
            for h in range(NH):
                slc = slice(h * HD, (h + 1) * HD)
                nc.vector.scalar_tensor_tensor(
                    o_t[:, slc], bc_a[:, slc], ztot[:, h:h + 1], o_t[:, slc],
                    Alu.mult, Alu.add)
                nc.vector.tensor_scalar(
                    o_t[:, slc], o_t[:, slc], zrec[:, h:h + 1], None, Alu.mult)

            def transpose6(src):
                outs = []
                for d in range(6):
                    pt = pss.tile([128, RR], f32, tag="ps_sm")
                    nc.tensor.transpose(pt[:], src[:, d * 128:(d + 1) * 128],
                                        ident[0:RR, 0:RR])
                    t = sm.tile([128, RR], f32r, tag=f"t64_{d}")
                    nc.vector.tensor_copy(t[:], pt[:])
                    outs.append(t)
                return outs

            oT = transpose6(o_t)
            ps_o = psb.tile([RR, D], f32, tag="psbig")
            for d in range(6):
                wot = pp.tile([128, D], f32r, tag=f"wq{d}")
                dma(out=wot[:], in_=wo_in[d * 128:(d + 1) * 128, :].bitcast(f32r))
                nc.tensor.matmul(ps_o[:, 0:512], oT[d][:], wot[:, 0:512],
                                 start=(d == 0), stop=(d == 5))
                nc.tensor.matmul(ps_o[:, 512:768], oT[d][:], wot[:, 512:768],
                                 start=(d == 0), stop=(d == 5))
            bc_b = sm.tile([RR, D], f32, tag="bc_b")
            nc.gpsimd.partition_broadcast(bc_a[:], vecs[:, 0:D])   # out_proj_b
            nc.gpsimd.partition_broadcast(bc_b[:], vecs[:, 6 * D:])   # cls
            x_t = sm.tile([RR, D], f32, tag="x_t")
            nc.vector.tensor_tensor(x_t[:], ps_o[:], bc_a[:], Alu.add)
            nc.vector.tensor_tensor(x_t[:], x_t[:], bc_b[:], Alu.add)

            def layer_norm(x, wrow, brow):
                mcol = sm.tile([RR, 1], f32, tag="ln_m")
                nc.vector.tensor_reduce(mcol[:], x[:], AX, Alu.add)
                nc.vector.tensor_scalar(mcol[:], mcol[:], 1.0 / float(D), None,
                                        Alu.mult)
                nc.vector.tensor_scalar(x[:], x[:], mcol[:], None, Alu.subtract)
                vsq = sm.tile([RR, D], f32, tag="sq64")
                vss = sm.tile([RR, 1], f32, tag="ln_ss")
                nc.scalar.activation(vsq[:], x[:], Act.Square, accum_out=vss[:])
                nc.vector.tensor_scalar(vss[:], vss[:], 1.0 / float(D), None,
                                        Alu.mult)
                nc.vector.tensor_scalar(vss[:], vss[:], 1e-5, None, Alu.add)
                nc.scalar.activation(vss[:], vss[:], Act.Sqrt)
                nc.vector.reciprocal(vss[:], vss[:])
                nc.vector.tensor_scalar(x[:], x[:], vss[:], None, Alu.mult)
                nc.gpsimd.partition_broadcast(bc_a[:], wrow)
                nc.gpsimd.partition_broadcast(bc_b[:], brow)
                nc.vector.tensor_tensor(x[:], x[:], bc_a[:], Alu.mult)
                nc.vector.tensor_tensor(x[:], x[:], bc_b[:], Alu.add)

            layer_norm(x_t, vecs[:, 2 * D:3 * D], vecs[:, 3 * D:4 * D])   # ln1_w, ln1_b

            xT = transpose6(x_t)
            l1ts = []
            for d in range(6):
                l1t = pp.tile([128, DFF], f32r, tag=f"wq{d}")
                dma(out=l1t[:], in_=l1_in[d * 128:(d + 1) * 128, :].bitcast(f32r))
                l1ts.append(l1t)
            h_t = sm.tile([RR, DFF], f32, tag="h_t")
            for grp in range(2):
                ps_h = psb.tile([RR, DFF // 2], f32, tag="psbig")
                for d in range(6):
                    base = grp * (DFF // 2)
                    for s in range(2):
                        nc.tensor.matmul(
                            ps_h[:, s * 512:(s + 1) * 512], xT[d][:],
                            l1ts[d][:, base + s * 512:base + (s + 1) * 512],
                            start=(d == 0), stop=(d == 5))
                nc.vector.tensor_copy(
                    h_t[:, grp * (DFF // 2):(grp + 1) * (DFF // 2)], ps_h[:])

            ps_f = psg.tile([RR, D], f32, tag="psg")
            for d in range(16):
                pt = pss.tile([128, RR], f32, tag="ps_sm")
                nc.tensor.transpose(pt[:], h_t[:, d * 128:(d + 1) * 128],
                                    ident[0:RR, 0:RR])
                gT = fs.tile([128, RR], f32r, tag="gT")
                nc.vector.tensor_scalar(gT[:], pt[:], b1c[:, d:d + 1], None, Alu.add)
                if GELU_SIM:
                    x3 = fs.tile([128, RR], f32, tag="gx3")
                    nc.vector.tensor_tensor(x3[:], gT[:], gT[:], Alu.mult)
                    nc.vector.tensor_tensor(x3[:], x3[:], gT[:], Alu.mult)
                    nc.vector.scalar_tensor_tensor(
                        x3[:], x3[:], 0.044715, gT[:], Alu.mult, Alu.add)
                    nc.scalar.activation(x3[:], x3[:], Act.Tanh,
                                         scale=0.7978845608028654)
                    nc.vector.tensor_scalar(x3[:], x3[:], 1.0, 0.5,
                                            Alu.add, Alu.mult)
                    nc.vector.tensor_tensor(gT[:], gT[:], x3[:], Alu.mult)
                else:
                    nc.scalar.activation(gT[:], gT[:], Act.Gelu)
                l2t = fs.tile([128, D], f32r, tag="fj")
                dma(out=l2t[:], in_=l2_in[d * 128:(d + 1) * 128, :].bitcast(f32r))
                nc.tensor.matmul(ps_f[:, 0:512], gT[:], l2t[:, 0:512],
                                 start=(d == 0), stop=(d == 15))
                nc.tensor.matmul(ps_f[:, 512:768], gT[:], l2t[:, 512:768],
                                 start=(d == 0), stop=(d == 15))
            nc.gpsimd.partition_broadcast(bc_a[:], vecs[:, D:2 * D])   # lin2_b
            fin = sm.tile([RR, D], f32, tag="v64a")
            nc.vector.tensor_tensor(fin[:], ps_f[:], bc_a[:], Alu.add)
            nc.vector.tensor_tensor(fin[:], fin[:], x_t[:], Alu.add)
            layer_norm(fin, vecs[:, 4 * D:5 * D], vecs[:, 5 * D:6 * D])   # ln2_w, ln2_b
            dma(out=out_p[:], in_=fin[:])

    nc.finalize()
    return nc


def _prep_inputs(F, T, cls_token, in_proj_w, in_proj_b, out_proj_w, out_proj_b,
                 lin1_w, lin1_b, lin2_w, lin2_b, ln1_w, ln1_b, ln2_w, ln2_b):
    f32 = np.float32
    cls = np.ascontiguousarray(np.asarray(cls_token).reshape(D).astype(f32))
    wqkv = np.ascontiguousarray(np.asarray(in_proj_w).astype(f32).T)
    wo = np.ascontiguousarray(np.asarray(out_proj_w).astype(f32).T)
    l1 = np.ascontiguousarray(np.asarray(lin1_w).astype(f32).T)
    l2 = np.ascontiguousarray(np.asarray(lin2_w).astype(f32).T)
    bqkv = np.ascontiguousarray(np.asarray(in_proj_b).astype(f32).reshape(1, 3 * D))
    b1c = np.ascontiguousarray(np.asarray(lin1_b).astype(f32).reshape(NCH, 128).T)
    vecs = np.zeros((7, D), f32)
    vecs[0] = out_proj_b
    vecs[1] = lin2_b
    vecs[2] = ln1_w
    vecs[3] = ln1_b
    vecs[4] = ln2_w
    vecs[5] = ln2_b
    vecs[6] = cls
    vecs = vecs.reshape(1, 7 * D)
    ident = np.eye(128, dtype=f32)
    consts = np.zeros((128, 2, RR), f32)
    consts[:, 0, :] = np.arange(RR)[None, :]
    consts[:, 1, :] = np.arange(RR)[None, :] - 65536.0
    F = np.asarray(F)
    T = np.asarray(T)
    in_maps = []
    for c in range(NCORE):
        fsh = np.ascontiguousarray(F[c * NLOC:(c + 1) * NLOC].astype(f32))
        ft = np.zeros((D, 128 * NCH_A), f32)
        ft[:, :NLOC] = fsh.T
        ft[:, NLOC] = cls
        tsh = np.ascontiguousarray(
            T[c * NLOC:(c + 1) * NLOC].astype(f32).reshape(NCH, 128, 2)
            .transpose(1, 2, 0))
        in_maps.append({
            "f": fsh, "ft": np.ascontiguousarray(ft), "tsh": tsh,
            "wqkv": wqkv, "wo": wo, "l1": l1, "l2": l2, "bqkv": bqkv,
            "b1c": b1c, "vecs": vecs, "ident": ident, "consts": consts,
        })
    return in_maps


def _install_trace_shim():
    """The image's antenv lacks axon_hooks; inject it + the ctypes NTFF hook."""
    import types, ctypes, contextlib
    try:
        from antenv.axon_hooks import get_axon_ntff_profile_hook  # noqa: F401
        return
    except ImportError:
        pass
    lib = ctypes.CDLL("/opt/axon/libaxon_pjrt.so")
    if not hasattr(lib, "axon_start_nrt_profile"):
        return
    lib.axon_start_nrt_profile.argtypes = [
        ctypes.POINTER(ctypes.c_int64), ctypes.c_size_t]
    lib.axon_start_nrt_profile.restype = ctypes.c_int64
    lib.axon_stop_nrt_profile.argtypes = [ctypes.c_char_p]
    lib.axon_stop_nrt_profile.restype = ctypes.c_int64

    @contextlib.contextmanager
    def _hook(output_dir, device_ids):
        import jax
        jax.devices()
        if device_ids:
            ids = (ctypes.c_int64 * len(device_ids))(*device_ids)
            rc = lib.axon_start_nrt_profile(ids, len(device_ids))
        else:
            rc = lib.axon_start_nrt_profile(None, 0)
        if rc != 0:
            raise RuntimeError(f"axon_start_nrt_profile rc={rc}")
        try:
            yield
        finally:
            n = lib.axon_stop_nrt_profile(str(output_dir).encode())
            print(f"ntff profile: {n} file(s) -> {output_dir}", file=sys.stderr)

    mod = types.ModuleType("antenv.axon_hooks")
    _h = [_hook]
    mod.get_axon_ntff_profile_hook = lambda: _h[0]
    mod.set_axon_ntff_profile_hook = lambda h: _h.__setitem__(0, h)
    sys.modules["antenv.axon_hooks"] = mod
    import antenv
    antenv.axon_hooks = mod
    import concourse.bass_utils as bu
    bu.upload_artifacts = lambda tmpdir: ""


def _run(inputs, trace=False):
    from concourse.bass_utils import run_bass_kernel_spmd
    if trace:
        _install_trace_shim()
    if "nc" not in _CACHE:
        _CACHE["nc"] = _build()
    nc = _CACHE["nc"]
    in_maps = _prep_inputs(**inputs)
    res = run_bass_kernel_spmd(nc, in_maps, core_ids=list(range(NCORE)),
                               trace=trace)
    out = np.asarray(res.results[0]["out"], dtype=np.float32)
    ri = np.concatenate([
        np.asarray(res.results[c]["ri"], dtype=np.float32).T.reshape(NLOC)
        for c in range(NCORE)])
    return (out, ri.astype(np.int32)), res


def kernel(**inputs):
    (out, ri), _ = _run(inputs, trace=False)
    return out, ri
